# revision 54
# baseline (speedup 1.0000x reference)
"""Multi-Query Attention kernel for 8x TRN2 NeuronCores (Bass/Tile).

Problem: x[B=2, L=2048, D=2048], Wq[2048,2048], Wk/Wv[128,2048] (MQA: one
shared K/V head), 16 query heads of dim 128.

Sharding: core c in [0,8): batch b = c//4, head-group g = c%4 (4 heads,
i.e. q-channels [512g, 512g+512)). K/V replicated per core (cheap).

Device-side layout strategy (everything "transposed" so that every matmul
contraction dim lands on SBUF partitions, with zero on-device transposes of
the big tensors):
  - host passes xT = x[b].T            [D, L]  (contraction dim D on rows)
  - host passes wqT/wkT/wvT = W.T      [D, out]
  - projections compute qT/kT/vT = W @ x.T = (x@W.T).T  -> [out_ch, L]
  - scores^T tile = (kT slice).T @ qT  -> [Lk, Lq]  (contraction d=128)
  - exp on ACT engine straight out of PSUM (scale fused), no max-subtract
    (inputs are small: |scores*scale| < ~6, exp is safe), OUTPUT IN BF16
  - out^T = (V block).T @ attn^T accumulated over Lk blocks in PSUM (V
    natural [L, d] in bf16 obtained via 16 cheap 128x128 PE transposes)
  - softmax denominator: DVE accumulates the bf16 exp tiles elementwise
    over the 16 Lk blocks (acc[k%128, q] = sum over blocks), then TWO small
    128x128-ones matmuls per pass reduce over partitions AND replicate the
    result r across partitions in PSUM (so normalization is a plain DVE
    multiply).  This keeps the partition-reduction almost entirely OFF the
    PE (the old scheme burned 131k PE cycles on full-size ones-matmuls).
  - 1/r via the fast custom-DVE reciprocal (5x faster than the exact one;
    ~18 good bits, way beyond what softmax needs)
  - final normalize multiply on DVE reads the AV PSUM banks directly (no
    ACT drain); results DMA'd out per head
  - host transposes outT [512, L] back and concatenates core outputs

Matmuls: scores/projections run float32r (full fp32 storage, reduced
precision multiply, 1 cycle/row at N>=512).  Post-softmax tensors (attn
weights, V) are bf16: same PE rate, half the SBUF traffic, and they let the
DVE accumulate the softmax denominator at its 2x 16-bit rate.

PSUM budget (8 banks of [128,512]f32): scores 2 bufs x [128,1024] (4 banks)
+ AV accumulator [128,1024] (2) + replicated-r [128,1024] (2) = 8 exactly.
"""

import os
from contextlib import ExitStack

import numpy as np

import concourse.bass as bass
import concourse.tile as tile
from concourse import bacc, masks, mybir
from concourse.bass_utils import run_bass_kernel_spmd

F32 = mybir.dt.float32
BF16 = mybir.dt.bfloat16
F16 = mybir.dt.float16
AF = mybir.ActivationFunctionType

B = 2
L = 2048
D = 2048  # d_model (contraction dim of projections)
HD = 128  # head dim
NH = 4  # heads per core
QC = NH * HD  # q-channels per core = 512
DC = D // 128  # d-model chunks of 128 = 16
NLT = 4  # l tiles of 512 (projection phase)
LKT = L // 128  # lk blocks of 128 = 16
NLQ = 4  # lq blocks of 512 (attention phase)
N_CORES = 8
SCALE = 1.0 / float(np.sqrt(HD))

# float32r: reduced-precision (tf32-like) matmul at full PE rate. Walrus
# requires every producer of an f32r-matmul operand to emit f32r, so all
# matmul-operand tiles are declared float32r and DRAM-side DMA APs are
# bitcast. Set BASS_MM_F32=1 to fall back to exact fp32 (4x slower on PE).
MM_DT = F32 if os.environ.get("BASS_MM_F32") else mybir.dt.float32r

# K/V projection sharding: each core projects only its 512-column l-slice of
# K/V, then an AllGather over the 4-core batch group reconstructs the full
# kT/vT (the old path recomputed them 4x). BASS_NO_CC=1 restores the
# replicated computation (no collectives).
# Measured on HW: the 4-core AllGather has ~85us latency from input-ready to
# gathered-data-landed (firmware launch dominated), which cannot hide under
# the ~60us of remaining Q-projection work — it LOST ~45us net. Default off.
USE_CC = bool(os.environ.get("BASS_USE_CC"))


def _dr(ap):
    # bitcast a DRAM-side fp32 AP for DMA into an MM_DT tile
    return ap.bitcast(MM_DT) if MM_DT != F32 else ap


def build_kernel(ctx: ExitStack, tc: tile.TileContext, xT, xkT, wT, bq, bk, bv, outT):
    nc = tc.nc

    persist = ctx.enter_context(tc.tile_pool(name="persist", bufs=1))
    # qT/kT in fp16: the scores matmuls keep their 1 cycle/row rate but the
    # per-matmul stationary load drops from the serialized ~185ns f32r path
    # to the ~95ns pipelined 2-byte path (f32r forbids standalone LDWEIGHTS).
    # Precision: Q/K are quantized AFTER the exact f32r projection; the
    # score error is ~0.1% of its std -> ~0.2% attn error, well in budget.
    qT = [persist.tile([128, L], F16, tag=f"qT{h}", name=f"qT{h}") for h in range(NH)]  # [d, l]
    kT = persist.tile([128, L], F16, tag="kT", name="kT")  # [d, l]
    vN = persist.tile([128, L], BF16, tag="vN", name="vN")  # block j: [:, 128j:+128] = V[128j:+128, :]
    ones_bf = persist.tile([128, 128], BF16, tag="ones_bf", name="ones_bf")
    ident = persist.tile([128, 128], BF16, tag="ident", name="ident")
    bq_sb = persist.tile([128, NH], F32, tag="bq", name="bq")
    bk_sb = persist.tile([128, 1], F32, tag="bk", name="bk")
    bv_sb = persist.tile([128, 1], F32, tag="bv", name="bv")

    nc.vector.memset(ones_bf[:], 1.0)
    masks.make_identity(nc, ident[:])
    # bias loads issued from the ACT engine's DGE so the SP queueing stays
    # free for the x / weight streams that gate the first matmul
    nc.scalar.dma_start(out=bq_sb[:], in_=bq)
    nc.scalar.dma_start(out=bk_sb[:], in_=bk)
    nc.scalar.dma_start(out=bv_sb[:], in_=bv)

    # ---------------- Phase A: PE warmup ----------------
    # The PE runs at 0.65/1.2 GHz until it has been continuously busy ~3us.
    # The first real matmul can't start until the first x/weight chunks land
    # (~13us in), so spend the DMA wait ramping the PE with throwaway
    # 128x128 matmuls (they also absorb the pstate penalty that would
    # otherwise tax the first ~dozen real matmuls).
    with tc.tile_pool(name="warm", bufs=1, space="PSUM") as wp:
        pw = wp.tile([128, 128], F32, tag="warm", name="pw")
        for _ in range(30):
            nc.tensor.matmul(pw[:], lhsT=ones_bf[:], rhs=ident[:], start=True, stop=True)

    # SBUF-side attention pools opened for the whole kernel: the projection
    # phase already computes scores+exp for the first two attention passes
    # (lk blocks 0-7 of lq=0, both head pairs) in its PE/ACT slack, which
    # removes 16 of the 128 exp instructions from the ACT-bound attention
    # phase.
    attp = ctx.enter_context(tc.tile_pool(name="att", bufs=6))
    accp = ctx.enter_context(tc.tile_pool(name="acc", bufs=2))
    finp = ctx.enter_context(tc.tile_pool(name="fin", bufs=2))
    eatp = ctx.enter_context(tc.tile_pool(name="eat", bufs=1))
    early_at = {}  # (p_idx, lk) -> exp'd bf16 tile kept until the AV sweep
    accs = {}  # p_idx -> DVE denominator accumulator

    # ---------------- Phase B: projections qT/kT/vT = W @ x^T ----------------
    # With USE_CC, each core projects K/V only for its own 512-column l-slice
    # (xkT, chosen by the host as slice c%4) right after the first Q l-tile,
    # then an AllGather over the 4-core batch group reassembles the full
    # kT/vT while the remaining Q projections run. This removes the 4x
    # replicated K/V compute (~20us of PE per core).
    vTg = persist.tile([128, L], BF16, tag="vTg", name="vTg") if USE_CC else None
    with (
        tc.tile_pool(name="wp", bufs=1) as wpp,
        tc.tile_pool(name="xt", bufs=16) as xtp,
        tc.tile_pool(name="xk", bufs=16) as xkp,
        tc.tile_pool(name="pj", bufs=1, space="PSUM") as pjp,
        tc.tile_pool(name="tpg", bufs=1, space="PSUM") as tpg,
        tc.tile_pool(name="esp", bufs=1, space="PSUM") as esp,
        tc.tile_pool(name="vt", bufs=1) as vtp,
        tc.tile_pool(name="dram", bufs=1, space="DRAM") as dramp,
    ):
        # one tile per d-chunk so each matmul waits on a single DMA sem.
        # Host concatenates [Wq | Wk | Wv].T into one [D, 768] tensor so a
        # chunk is ONE DMA: the SP queue can only issue ~1.6 DMAs/us, and at
        # 4 issues per chunk (old layout) the PE starves during the first
        # l-tile while all the weights stream in.
        w_ch = [wpp.tile([128, QC + 2 * HD], MM_DT, tag=f"wc{k}", name=f"wc{k}") for k in range(DC)]
        if USE_CC:
            xk_ch = [xkp.tile([128, 512], MM_DT, tag="xk", name="xk") for _ in range(DC)]
        else:
            vT = [vtp.tile([128, 512], BF16, tag=f"vT{t}", name=f"vT{t}") for t in range(NLT)]

        def emit_early_sc(p_idx, lk):
            """Scores+exp+denominator-add for (pass p_idx, lk block) during
            the projection phase, using the 2-bank esp PSUM pool. Paced >=2
            chunks apart so the single-buffered ss never stalls the PE."""
            hp = p_idx % 2
            ks = slice(lk * 128, (lk + 1) * 128)
            ss = esp.tile([128, 1024], F32, tag="ess", name="ess")
            for j in range(2):
                nc.tensor.matmul(
                    ss[:, j * 512:(j + 1) * 512],
                    lhsT=kT[:, ks],
                    rhs=qT[2 * hp + j][:, 0:512],
                    start=True,
                    stop=True,
                )
            at = eatp.tile([128, 1024], BF16, tag=f"eat{p_idx}_{lk}", name=f"eat{p_idx}_{lk}")
            nc.scalar.activation(at[:], ss[:], AF.Exp, scale=SCALE)
            if p_idx not in accs:
                accs[p_idx] = accp.tile([128, 1024], BF16, tag="acc", name="acc")
                nc.vector.tensor_copy(accs[p_idx][:], at[:])
            else:
                nc.vector.tensor_add(accs[p_idx][:], accs[p_idx][:], at[:])
            early_at[(p_idx, lk)] = at

        def emit_tp(t_lt, jj):
            """One V transpose, single-banked; callers space them out so the
            drain-latency chain hides behind projection matmuls."""
            j = t_lt * 4 + jj
            pt = tpg.tile([128, 128], BF16, tag="tp", name="tp")
            nc.tensor.transpose(pt[:], vT[t_lt][:, jj * 128:(jj + 1) * 128], ident[:])
            nc.scalar.activation(vN[:, j * 128:(j + 1) * 128], pt[:], AF.Identity)

        # (lt, phase, k) -> (p_idx, lk): blocks lk 0-7 of attention passes 0
        # and 1, each gated on kT columns already drained at that point
        sc_sched = {
            (1, 1, 7): (0, 0), (1, 1, 9): (0, 1), (1, 1, 11): (0, 2), (1, 1, 13): (0, 3),
            (1, 2, 4): (0, 4), (1, 2, 12): (0, 5),
            (2, 1, 5): (0, 6), (2, 1, 7): (0, 7), (2, 1, 9): (1, 0), (2, 1, 11): (1, 1),
            (2, 1, 13): (1, 2), (2, 1, 15): (1, 3),
            (3, 1, 5): (1, 4), (3, 1, 7): (1, 5), (3, 1, 9): (1, 6), (3, 1, 11): (1, 7),
        }
        # (lt, phase, k) -> V l-tile whose 4 transposes spread over that loop
        tr_slots = {(1, 1): 0, (2, 1): 1, (3, 1): 2, (3, 2): 3}
        tr_ks = {1: (1, 4, 8, 12), 2: (2, 6, 10, 14)}

        for lt in range(NLT):
            ls = slice(lt * 512, (lt + 1) * 512)
            # concurrent PSUM accumulation groups: Q0..Q3 (+ shared K/V)
            psq = [pjp.tile([128, 512], F32, tag=f"pjq{t}", name=f"pjq{t}") for t in range(NH)]
            if USE_CC and lt == 0:
                # K/V-slice x chunks: issued on the ACT engine's DGE
                # (idle during projections), consumed right after l-tile 0
                for k in range(DC):
                    nc.scalar.dma_start(out=xk_ch[k][:], in_=_dr(xkT[k * 128:(k + 1) * 128, :]))
            # K and V share ONE PSUM tag via two sweeps: phase 1 does Q+K
            # (Q+V for the last tile), phase 2 re-streams the SBUF-resident
            # x chunks through the other weight. This frees a PSUM bank for
            # the early-attention scores pool. For the last tile K goes in
            # phase 2 but its drain still lands before the first attention
            # scores that need it.
            KCOL = slice(QC, QC + HD)
            VCOL = slice(QC + HD, QC + 2 * HD)
            ph1_col, ph2_col = (KCOL, VCOL) if lt < NLT - 1 else (VCOL, KCOL)
            if not USE_CC:
                pskv = pjp.tile([128, 512], F32, tag="pjkv", name="pjkv")
            xcs = []
            for k in range(DC):
                xc = xtp.tile([128, 512], MM_DT, tag="xt", name="xt")
                xcs.append(xc)
                if lt == 0 and k == 0:
                    # split the very first transfers across queues so the
                    # first matmul isn't serialized behind one queue's ramp
                    for q in range(4):
                        fs = slice(q * 128, (q + 1) * 128)
                        nc.sync.dma_start(out=xc[:, fs], in_=_dr(xT[0:128, q * 128:(q + 1) * 128]))
                    nc.sync.dma_start(out=w_ch[0][:], in_=_dr(wT[0:128, :]))
                else:
                    nc.sync.dma_start(out=xc[:], in_=_dr(xT[k * 128:(k + 1) * 128, ls]))
                    if lt == 0 and k > 0:
                        nc.sync.dma_start(out=w_ch[k][:], in_=_dr(wT[k * 128:(k + 1) * 128, :]))
                st = k == 0
                sp = k == DC - 1
                for t in range(NH):
                    nc.tensor.matmul(
                        psq[t][:],
                        lhsT=w_ch[k][:, t * 128:(t + 1) * 128],
                        rhs=xc[:],
                        start=st,
                        stop=sp,
                    )
                if not USE_CC:
                    nc.tensor.matmul(pskv[:], lhsT=w_ch[k][:, ph1_col], rhs=xc[:], start=st, stop=sp)
                    if tr_slots.get((lt, 1)) is not None and k in tr_ks[1]:
                        emit_tp(tr_slots[(lt, 1)], tr_ks[1].index(k))
                    if (lt, 1, k) in sc_sched:
                        emit_early_sc(*sc_sched[(lt, 1, k)])
            if not USE_CC:
                if lt < NLT - 1:
                    nc.scalar.activation(kT[:, ls], pskv[:], AF.Identity, bias=bk_sb[:, 0:1])
                else:
                    nc.scalar.activation(vT[lt][:], pskv[:], AF.Identity, bias=bv_sb[:, 0:1])
            for t in range(NH):
                nc.scalar.activation(qT[t][:, ls], psq[t][:], AF.Identity, bias=bq_sb[:, t:t + 1])
            if not USE_CC:
                # phase 2: the other K/V projection, re-streaming resident x
                pskv2 = pjp.tile([128, 512], F32, tag="pjkv", name="pjkv2")
                for k in range(DC):
                    nc.tensor.matmul(
                        pskv2[:], lhsT=w_ch[k][:, ph2_col], rhs=xcs[k][:],
                        start=k == 0, stop=k == DC - 1,
                    )
                    if tr_slots.get((lt, 2)) is not None and k in tr_ks[2]:
                        emit_tp(tr_slots[(lt, 2)], tr_ks[2].index(k))
                    if (lt, 2, k) in sc_sched:
                        emit_early_sc(*sc_sched[(lt, 2, k)])
                if lt < NLT - 1:
                    nc.scalar.activation(vT[lt][:], pskv2[:], AF.Identity, bias=bv_sb[:, 0:1])
                else:
                    nc.scalar.activation(kT[:, ls], pskv2[:], AF.Identity, bias=bk_sb[:, 0:1])
            if USE_CC and lt == 0:
                # ---- K/V slice projection + AllGather ----
                with tc.tile_pool(name="kvp", bufs=1, space="PSUM") as kvp:
                    psk = kvp.tile([128, 512], F32, tag="kvk", name="kvk")
                    psv = kvp.tile([128, 512], F32, tag="kvv", name="kvv")
                    for k in range(DC):
                        st = k == 0
                        sp = k == DC - 1
                        nc.tensor.matmul(psk[:], lhsT=w_ch[k][:, QC:QC + HD], rhs=xk_ch[k][:], start=st, stop=sp)
                        nc.tensor.matmul(psv[:], lhsT=w_ch[k][:, QC + HD:], rhs=xk_ch[k][:], start=st, stop=sp)
                    ksb = vtp.tile([128, 512], F16, tag="ksb", name="ksb")
                    vsb = vtp.tile([128, 512], BF16, tag="vsb", name="vsb")
                    nc.scalar.activation(ksb[:], psk[:], AF.Identity, bias=bk_sb[:, 0:1])
                    nc.scalar.activation(vsb[:], psv[:], AF.Identity, bias=bv_sb[:, 0:1])
                kin = dramp.tile([128, 512], F16, tag="kin", name="kin")
                vin = dramp.tile([128, 512], BF16, tag="vin", name="vin")
                kout = dramp.tile([4, 128, 512], F16, tag="kout", name="kout")
                vout = dramp.tile([4, 128, 512], BF16, tag="vout", name="vout")
                nc.scalar.dma_start(out=kin, in_=ksb[:])
                nc.scalar.dma_start(out=vin, in_=vsb[:])
                nc.gpsimd.collective_compute(
                    "AllGather",
                    mybir.AluOpType.bypass,
                    replica_groups=[[0, 1, 2, 3], [4, 5, 6, 7]],
                    ins=[kin.opt()],
                    outs=[kout.opt()],
                )
                nc.gpsimd.collective_compute(
                    "AllGather",
                    mybir.AluOpType.bypass,
                    replica_groups=[[0, 1, 2, 3], [4, 5, 6, 7]],
                    ins=[vin.opt()],
                    outs=[vout.opt()],
                )
                for g in range(4):
                    gs = slice(g * 512, (g + 1) * 512)
                    nc.sync.dma_start(out=kT[:, gs], in_=kout[g:g + 1])
                    nc.sync.dma_start(out=vTg[:, gs], in_=vout[g:g + 1])

        if USE_CC:
            # V natural-layout transposes from the gathered vT
            for j in range(L // 128):
                pt = tpg.tile([128, 128], BF16, tag="tp", name="tp")
                nc.tensor.transpose(pt[:], vTg[:, j * 128:(j + 1) * 128], ident[:])
                nc.scalar.activation(vN[:, j * 128:(j + 1) * 128], pt[:], AF.Identity)

    # ---------------- Phase D: attention ----------------
    # Per pass (head-pair hp x query tile lq): 16 Lk blocks. PE does 2
    # scores matmuls + 2 AV matmuls per block (512-free each, PSUM bank
    # limit); ACT exps the [128,1024] scores tile to bf16; DVE accumulates
    # the bf16 exp tiles into acc. At pass end two 128-wide ones-matmuls
    # turn acc into a partition-replicated denominator, DVE inverts it and
    # scales the AV accumulator straight out of PSUM.
    with (
        tc.tile_pool(name="sps", bufs=2, space="PSUM") as sps,  # 2 x [128,1024] = 4 banks
        tc.tile_pool(name="avp", bufs=1, space="PSUM") as avp,  # [128,1024] = 2 banks
        tc.tile_pool(name="rvp", bufs=1, space="PSUM") as rvp,  # [128,1024] = 2 banks
    ):
        for lq in range(NLQ):
            qs = slice(lq * 512, (lq + 1) * 512)
            for hp in range(2):  # head pairs
                p_idx = lq * 2 + hp
                psA = avp.tile([128, 1024], F32, tag="av", name="av")
                if p_idx in accs:
                    acc = accs[p_idx]
                    at_of = {lk: early_at[(p_idx, lk)] for lk in range(8)}
                    start_lk = 8
                else:
                    acc = accp.tile([128, 1024], BF16, tag="acc", name="acc")
                    at_of = {}
                    start_lk = 0

                def emit_sc(lk):
                    ss = sps.tile([128, 1024], F32, tag="sps", name="sps")
                    for j in range(2):
                        nc.tensor.matmul(
                            ss[:, j * 512:(j + 1) * 512],
                            lhsT=kT[:, lk * 128:(lk + 1) * 128],
                            rhs=qT[2 * hp + j][:, qs],
                            start=True,
                            stop=True,
                        )
                    at = attp.tile([128, 1024], BF16, tag="att", name="att")
                    nc.scalar.activation(at[:], ss[:], AF.Exp, scale=SCALE)
                    # softmax denominator partials on the DVE (2x bf16 rate)
                    if lk == 0:
                        nc.vector.tensor_copy(acc[:], at[:])
                    else:
                        nc.vector.tensor_add(acc[:], acc[:], at[:])
                    at_of[lk] = at

                def emit_av(lk):
                    for j in range(2):
                        nc.tensor.matmul(
                            psA[:, j * 512:(j + 1) * 512],
                            lhsT=vN[:, lk * 128:(lk + 1) * 128],
                            rhs=at_of[lk][:, j * 512:(j + 1) * 512],
                            start=lk == 0,
                            stop=lk == LKT - 1,
                        )

                if start_lk == 0:
                    # full pass: interleave, AV trailing exp by 2 blocks
                    pend = []
                    for lk in range(LKT):
                        emit_sc(lk)
                        pend.append(lk)
                        if len(pend) > 2:
                            emit_av(pend.pop(0))
                    while pend:
                        emit_av(pend.pop(0))
                else:
                    # first two passes: lk 0-7 already exp'd during the
                    # projections — stream their AVs while the missing
                    # scores trickle in at the ACT's exp cadence
                    emit_sc(8)
                    emit_sc(9)
                    next_sc = 10
                    for lk in range(LKT):
                        emit_av(lk)
                        if lk % 2 == 1 and next_sc < LKT:
                            emit_sc(next_sc)
                            next_sc += 1
                # partition-reduce + replicate the denominator: r[*, q] =
                # sum_k acc[k, q] for every output partition
                psR = rvp.tile([128, 1024], F32, tag="rv", name="rv")
                for j in range(2):
                    nc.tensor.matmul(
                        psR[:, j * 512:(j + 1) * 512],
                        lhsT=ones_bf[:],
                        rhs=acc[:, j * 512:(j + 1) * 512],
                        start=True,
                        stop=True,
                    )
                rinv = finp.tile([128, 1024], F32, tag="rinv", name="rinv")
                nc.vector.reciprocal_approx_fast(rinv[:], psR[:])
                ot = finp.tile([128, 1024], F32, tag="ot", name="ot")
                nc.vector.tensor_mul(ot[:], psA[:], rinv[:])
                for j in range(2):
                    h = 2 * hp + j
                    nc.sync.dma_start(
                        out=outT[h * 128:(h + 1) * 128, qs],
                        in_=ot[:, j * 512:(j + 1) * 512],
                    )


_NC_CACHE = None


def build_nc():
    global _NC_CACHE
    if _NC_CACHE is not None:
        return _NC_CACHE
    nc = bacc.Bacc("TRN2", target_bir_lowering=False, debug=False)
    xT = nc.dram_tensor("xT", [D, L], F32, kind="ExternalInput").ap()
    xkT = nc.dram_tensor("xkT", [D, 512], F32, kind="ExternalInput").ap()
    wT = nc.dram_tensor("wT", [D, QC + 2 * HD], F32, kind="ExternalInput").ap()
    bq = nc.dram_tensor("bq", [128, NH], F32, kind="ExternalInput").ap()
    bk = nc.dram_tensor("bk", [128, 1], F32, kind="ExternalInput").ap()
    bv = nc.dram_tensor("bv", [128, 1], F32, kind="ExternalInput").ap()
    outT = nc.dram_tensor("outT", [QC, L], F32, kind="ExternalOutput").ap()
    with tile.TileContext(nc) as tc, ExitStack() as ctx:
        build_kernel(ctx, tc, xT, xkT, wT, bq, bk, bv, outT)
    nc.compile()
    _NC_CACHE = nc
    return nc


def make_in_maps(x, Wq_w, Wq_b, Wk_w, Wk_b, Wv_w, Wv_b):
    """Host-side sharding/relayout. Returns one input map per core."""
    x = np.asarray(x, dtype=np.float32)
    Wq_w = np.asarray(Wq_w, dtype=np.float32)
    Wq_b = np.asarray(Wq_b, dtype=np.float32)
    Wk_w = np.asarray(Wk_w, dtype=np.float32)
    Wk_b = np.asarray(Wk_b, dtype=np.float32)
    Wv_w = np.asarray(Wv_w, dtype=np.float32)
    Wv_b = np.asarray(Wv_b, dtype=np.float32)

    xTs = [np.ascontiguousarray(x[b].T) for b in range(B)]
    wkvT = np.concatenate([Wk_w.T, Wv_w.T], axis=1)  # [D, 256]
    bk = np.ascontiguousarray(Wk_b.reshape(128, 1))
    bv = np.ascontiguousarray(Wv_b.reshape(128, 1))
    in_maps = []
    for c in range(N_CORES):
        b, g = divmod(c, B * 2)  # b = c // 4, g = c % 4
        # one contiguous [D, 768] weight tensor: [Wq_g | Wk | Wv].T
        wT_g = np.ascontiguousarray(
            np.concatenate([Wq_w[g * QC:(g + 1) * QC, :].T, wkvT], axis=1)
        )
        bq_g = np.ascontiguousarray(Wq_b[g * QC:(g + 1) * QC].reshape(NH, 128).T)
        in_maps.append(
            {
                "xT": xTs[b],
                "xkT": np.ascontiguousarray(xTs[b][:, g * 512:(g + 1) * 512]),
                "wT": wT_g,
                "bq": bq_g,
                "bk": bk,
                "bv": bv,
            }
        )
    return in_maps


def assemble_output(results):
    out = np.empty((B, L, D), dtype=np.float32)
    for c in range(N_CORES):
        b, g = divmod(c, B * 2)
        out[b, :, g * QC:(g + 1) * QC] = results[c]["outT"].T
    return out


def kernel(**inputs) -> np.ndarray:
    nc = build_nc()
    in_maps = make_in_maps(**inputs)
    res = run_bass_kernel_spmd(nc, in_maps, core_ids=list(range(N_CORES)))
    return assemble_output(res.results)


# revision 55
# speedup vs baseline: 1.0247x; 1.0247x over previous
"""Multi-Query Attention kernel for 8x TRN2 NeuronCores (Bass/Tile).

Problem: x[B=2, L=2048, D=2048], Wq[2048,2048], Wk/Wv[128,2048] (MQA: one
shared K/V head), 16 query heads of dim 128.

Sharding: core c in [0,8): batch b = c//4, head-group g = c%4 (4 heads,
i.e. q-channels [512g, 512g+512)). K/V replicated per core (cheap).

Device-side layout strategy (everything "transposed" so that every matmul
contraction dim lands on SBUF partitions, with zero on-device transposes of
the big tensors):
  - host passes xT = x[b].T            [D, L]  (contraction dim D on rows)
  - host passes wqT/wkT/wvT = W.T      [D, out]
  - projections compute qT/kT/vT = W @ x.T = (x@W.T).T  -> [out_ch, L]
  - scores^T tile = (kT slice).T @ qT  -> [Lk, Lq]  (contraction d=128)
  - exp on ACT engine straight out of PSUM (scale fused), no max-subtract
    (inputs are small: |scores*scale| < ~6, exp is safe), OUTPUT IN BF16
  - out^T = (V block).T @ attn^T accumulated over Lk blocks in PSUM (V
    natural [L, d] in bf16 obtained via 16 cheap 128x128 PE transposes)
  - softmax denominator: DVE accumulates the bf16 exp tiles elementwise
    over the 16 Lk blocks (acc[k%128, q] = sum over blocks), then TWO small
    128x128-ones matmuls per pass reduce over partitions AND replicate the
    result r across partitions in PSUM (so normalization is a plain DVE
    multiply).  This keeps the partition-reduction almost entirely OFF the
    PE (the old scheme burned 131k PE cycles on full-size ones-matmuls).
  - 1/r via the fast custom-DVE reciprocal (5x faster than the exact one;
    ~18 good bits, way beyond what softmax needs)
  - final normalize multiply on DVE reads the AV PSUM banks directly (no
    ACT drain); results DMA'd out per head
  - host transposes outT [512, L] back and concatenates core outputs

Matmuls: scores/projections run float32r (full fp32 storage, reduced
precision multiply, 1 cycle/row at N>=512).  Post-softmax tensors (attn
weights, V) are bf16: same PE rate, half the SBUF traffic, and they let the
DVE accumulate the softmax denominator at its 2x 16-bit rate.

PSUM budget (8 banks of [128,512]f32): scores 2 bufs x [128,1024] (4 banks)
+ AV accumulator [128,1024] (2) + replicated-r [128,1024] (2) = 8 exactly.
"""

import os
from contextlib import ExitStack

import numpy as np

import concourse.bass as bass
import concourse.tile as tile
from concourse import bacc, masks, mybir
from concourse.bass_utils import run_bass_kernel_spmd

F32 = mybir.dt.float32
BF16 = mybir.dt.bfloat16
F16 = mybir.dt.float16
AF = mybir.ActivationFunctionType

B = 2
L = 2048
D = 2048  # d_model (contraction dim of projections)
HD = 128  # head dim
NH = 4  # heads per core
QC = NH * HD  # q-channels per core = 512
DC = D // 128  # d-model chunks of 128 = 16
NLT = 4  # l tiles of 512 (projection phase)
LKT = L // 128  # lk blocks of 128 = 16
NLQ = 4  # lq blocks of 512 (attention phase)
N_CORES = 8
SCALE = 1.0 / float(np.sqrt(HD))

# float32r: reduced-precision (tf32-like) matmul at full PE rate. Walrus
# requires every producer of an f32r-matmul operand to emit f32r, so all
# matmul-operand tiles are declared float32r and DRAM-side DMA APs are
# bitcast. Set BASS_MM_F32=1 to fall back to exact fp32 (4x slower on PE).
MM_DT = F32 if os.environ.get("BASS_MM_F32") else mybir.dt.float32r

# K/V projection sharding: each core projects only its 512-column l-slice of
# K/V, then an AllGather over the 4-core batch group reconstructs the full
# kT/vT (the old path recomputed them 4x). BASS_NO_CC=1 restores the
# replicated computation (no collectives).
# Measured on HW: the 4-core AllGather has ~85us latency from input-ready to
# gathered-data-landed (firmware launch dominated), which cannot hide under
# the ~60us of remaining Q-projection work — it LOST ~45us net. Default off.
USE_CC = bool(os.environ.get("BASS_USE_CC"))


def _dr(ap):
    # bitcast a DRAM-side fp32 AP for DMA into an MM_DT tile
    return ap.bitcast(MM_DT) if MM_DT != F32 else ap


def build_kernel(ctx: ExitStack, tc: tile.TileContext, xT, xkT, wT, bq, bk, bv, outT):
    nc = tc.nc

    persist = ctx.enter_context(tc.tile_pool(name="persist", bufs=1))
    # qT/kT in fp16: the scores matmuls keep their 1 cycle/row rate but the
    # per-matmul stationary load drops from the serialized ~185ns f32r path
    # to the ~95ns pipelined 2-byte path (f32r forbids standalone LDWEIGHTS).
    # Precision: Q/K are quantized AFTER the exact f32r projection; the
    # score error is ~0.1% of its std -> ~0.2% attn error, well in budget.
    qT = [persist.tile([128, L], F16, tag=f"qT{h}", name=f"qT{h}") for h in range(NH)]  # [d, l]
    kT = persist.tile([128, L], F16, tag="kT", name="kT")  # [d, l]
    vN = persist.tile([128, L], BF16, tag="vN", name="vN")  # block j: [:, 128j:+128] = V[128j:+128, :]
    ones_bf = persist.tile([128, 128], BF16, tag="ones_bf", name="ones_bf")
    ident = persist.tile([128, 128], BF16, tag="ident", name="ident")
    bq_sb = persist.tile([128, NH], F32, tag="bq", name="bq")
    bk_sb = persist.tile([128, 1], F32, tag="bk", name="bk")
    bv_sb = persist.tile([128, 1], F32, tag="bv", name="bv")

    nc.vector.memset(ones_bf[:], 1.0)
    masks.make_identity(nc, ident[:])
    # bias loads issued from the ACT engine's DGE so the SP queueing stays
    # free for the x / weight streams that gate the first matmul
    nc.scalar.dma_start(out=bq_sb[:], in_=bq)
    nc.scalar.dma_start(out=bk_sb[:], in_=bk)
    nc.scalar.dma_start(out=bv_sb[:], in_=bv)

    # ---------------- Phase A: PE warmup ----------------
    # The PE runs at 0.65/1.2 GHz until it has been continuously busy ~3us.
    # The first real matmul can't start until the first x/weight chunks land
    # (~13us in), so spend the DMA wait ramping the PE with throwaway
    # 128x128 matmuls (they also absorb the pstate penalty that would
    # otherwise tax the first ~dozen real matmuls).
    with tc.tile_pool(name="warm", bufs=1, space="PSUM") as wp:
        pw = wp.tile([128, 128], F32, tag="warm", name="pw")
        for _ in range(30):
            nc.tensor.matmul(pw[:], lhsT=ones_bf[:], rhs=ident[:], start=True, stop=True)

    # SBUF-side attention pools opened for the whole kernel: the projection
    # phase already computes scores+exp for the first two attention passes
    # (lk blocks 0-7 of lq=0, both head pairs) in its PE/ACT slack, which
    # removes 16 of the 128 exp instructions from the ACT-bound attention
    # phase.
    attp = ctx.enter_context(tc.tile_pool(name="att", bufs=6))
    accp = ctx.enter_context(tc.tile_pool(name="acc", bufs=2))
    finp = ctx.enter_context(tc.tile_pool(name="fin", bufs=2))
    eatp = ctx.enter_context(tc.tile_pool(name="eat", bufs=1))
    early_at = {}  # (p_idx, lk) -> exp'd bf16 tile kept until the AV sweep
    accs = {}  # p_idx -> DVE denominator accumulator

    # ---------------- Phase B: projections qT/kT/vT = W @ x^T ----------------
    # With USE_CC, each core projects K/V only for its own 512-column l-slice
    # (xkT, chosen by the host as slice c%4) right after the first Q l-tile,
    # then an AllGather over the 4-core batch group reassembles the full
    # kT/vT while the remaining Q projections run. This removes the 4x
    # replicated K/V compute (~20us of PE per core).
    vTg = persist.tile([128, L], BF16, tag="vTg", name="vTg") if USE_CC else None
    with (
        tc.tile_pool(name="wp", bufs=1) as wpp,
        tc.tile_pool(name="xt", bufs=6) as xtp,
        tc.tile_pool(name="xk", bufs=16) as xkp,
        tc.tile_pool(name="pj", bufs=1, space="PSUM") as pjp,
        tc.tile_pool(name="tpg", bufs=2, space="PSUM") as tpg,
        tc.tile_pool(name="vt", bufs=1) as vtp,
        tc.tile_pool(name="dram", bufs=1, space="DRAM") as dramp,
    ):
        # one tile per d-chunk so each matmul waits on a single DMA sem.
        # Host concatenates [Wq | Wk | Wv].T into one [D, 768] tensor so a
        # chunk is ONE DMA: the SP queue can only issue ~1.6 DMAs/us, and at
        # 4 issues per chunk (old layout) the PE starves during the first
        # l-tile while all the weights stream in.
        w_ch = [wpp.tile([128, QC + 2 * HD], MM_DT, tag=f"wc{k}", name=f"wc{k}") for k in range(DC)]
        if USE_CC:
            xk_ch = [xkp.tile([128, 512], MM_DT, tag="xk", name="xk") for _ in range(DC)]
        else:
            vT = [vtp.tile([128, 512], BF16, tag=f"vT{t}", name=f"vT{t}") for t in range(NLT)]

        # Early scores-during-projection experiment: needs a 2-bank PSUM
        # pool that conflicts with the V-transpose pool; the DMA-transpose
        # route that would free those banks corrupts data when issued from
        # SP and clogs the sequencer for 1.24us/op when issued from ACT.
        # Disabled.
        early_sched = {}

        for lt in range(NLT):
            ls = slice(lt * 512, (lt + 1) * 512)
            # concurrent PSUM accumulation groups: Q0..Q3 (+K, V w/o CC)
            psq = [pjp.tile([128, 512], F32, tag=f"pjq{t}", name=f"pjq{t}") for t in range(NH)]
            if not USE_CC:
                psk = pjp.tile([128, 512], F32, tag="pjk", name="pjk")
                psv = pjp.tile([128, 512], F32, tag="pjv", name="pjv")
            if USE_CC and lt == 0:
                # K/V-slice x chunks: issued on the ACT engine's DGE
                # (idle during projections), consumed right after l-tile 0
                for k in range(DC):
                    nc.scalar.dma_start(out=xk_ch[k][:], in_=_dr(xkT[k * 128:(k + 1) * 128, :]))
            for k in range(DC):
                xc = xtp.tile([128, 512], MM_DT, tag="xt", name="xt")
                if lt == 0 and k == 0:
                    # split the very first transfers across queues so the
                    # first matmul isn't serialized behind one queue's ramp
                    for q in range(4):
                        fs = slice(q * 128, (q + 1) * 128)
                        nc.sync.dma_start(out=xc[:, fs], in_=_dr(xT[0:128, q * 128:(q + 1) * 128]))
                    nc.sync.dma_start(out=w_ch[0][:], in_=_dr(wT[0:128, :]))
                else:
                    nc.sync.dma_start(out=xc[:], in_=_dr(xT[k * 128:(k + 1) * 128, ls]))
                    if lt == 0 and k > 0:
                        nc.sync.dma_start(out=w_ch[k][:], in_=_dr(wT[k * 128:(k + 1) * 128, :]))
                st = k == 0
                sp = k == DC - 1
                for t in range(NH):
                    nc.tensor.matmul(
                        psq[t][:],
                        lhsT=w_ch[k][:, t * 128:(t + 1) * 128],
                        rhs=xc[:],
                        start=st,
                        stop=sp,
                    )
                if not USE_CC:
                    nc.tensor.matmul(psk[:], lhsT=w_ch[k][:, QC:QC + HD], rhs=xc[:], start=st, stop=sp)
                    nc.tensor.matmul(psv[:], lhsT=w_ch[k][:, QC + HD:], rhs=xc[:], start=st, stop=sp)
            if not USE_CC:
                # vT first: the PE transposes below consume it and sit before
                # everything else in the in-order PE queue
                nc.scalar.activation(vT[lt][:], psv[:], AF.Identity, bias=bv_sb[:, 0:1])
                nc.scalar.activation(kT[:, ls], psk[:], AF.Identity, bias=bk_sb[:, 0:1])
            for t in range(NH):
                nc.scalar.activation(qT[t][:, ls], psq[t][:], AF.Identity, bias=bq_sb[:, t:t + 1])
            if not USE_CC:
                # transpose this l-tile of V to natural layout right away.
                # (XBAR DMA-transpose was tried and rejected: ~1.24us of
                # issuing-sequencer occupancy from ACT, and silently corrupt
                # data when issued from SP.)
                for jj in range(4):
                    j = lt * 4 + jj
                    pt = tpg.tile([128, 128], BF16, tag="tp", name="tp")
                    nc.tensor.transpose(pt[:], vT[lt][:, jj * 128:(jj + 1) * 128], ident[:])
                    nc.scalar.activation(vN[:, j * 128:(j + 1) * 128], pt[:], AF.Identity)
            if USE_CC and lt == 0:
                # ---- K/V slice projection + AllGather ----
                with tc.tile_pool(name="kvp", bufs=1, space="PSUM") as kvp:
                    psk = kvp.tile([128, 512], F32, tag="kvk", name="kvk")
                    psv = kvp.tile([128, 512], F32, tag="kvv", name="kvv")
                    for k in range(DC):
                        st = k == 0
                        sp = k == DC - 1
                        nc.tensor.matmul(psk[:], lhsT=w_ch[k][:, QC:QC + HD], rhs=xk_ch[k][:], start=st, stop=sp)
                        nc.tensor.matmul(psv[:], lhsT=w_ch[k][:, QC + HD:], rhs=xk_ch[k][:], start=st, stop=sp)
                    ksb = vtp.tile([128, 512], F16, tag="ksb", name="ksb")
                    vsb = vtp.tile([128, 512], BF16, tag="vsb", name="vsb")
                    nc.scalar.activation(ksb[:], psk[:], AF.Identity, bias=bk_sb[:, 0:1])
                    nc.scalar.activation(vsb[:], psv[:], AF.Identity, bias=bv_sb[:, 0:1])
                kin = dramp.tile([128, 512], F16, tag="kin", name="kin")
                vin = dramp.tile([128, 512], BF16, tag="vin", name="vin")
                kout = dramp.tile([4, 128, 512], F16, tag="kout", name="kout")
                vout = dramp.tile([4, 128, 512], BF16, tag="vout", name="vout")
                nc.scalar.dma_start(out=kin, in_=ksb[:])
                nc.scalar.dma_start(out=vin, in_=vsb[:])
                nc.gpsimd.collective_compute(
                    "AllGather",
                    mybir.AluOpType.bypass,
                    replica_groups=[[0, 1, 2, 3], [4, 5, 6, 7]],
                    ins=[kin.opt()],
                    outs=[kout.opt()],
                )
                nc.gpsimd.collective_compute(
                    "AllGather",
                    mybir.AluOpType.bypass,
                    replica_groups=[[0, 1, 2, 3], [4, 5, 6, 7]],
                    ins=[vin.opt()],
                    outs=[vout.opt()],
                )
                for g in range(4):
                    gs = slice(g * 512, (g + 1) * 512)
                    nc.sync.dma_start(out=kT[:, gs], in_=kout[g:g + 1])
                    nc.sync.dma_start(out=vTg[:, gs], in_=vout[g:g + 1])

        if USE_CC:
            # V natural-layout transposes from the gathered vT
            for j in range(L // 128):
                pt = tpg.tile([128, 128], BF16, tag="tp", name="tp")
                nc.tensor.transpose(pt[:], vTg[:, j * 128:(j + 1) * 128], ident[:])
                nc.scalar.activation(vN[:, j * 128:(j + 1) * 128], pt[:], AF.Identity)

    # ---------------- Phase D: attention ----------------
    # Per pass (head-pair hp x query tile lq): 16 Lk blocks. PE does 2
    # scores matmuls + 2 AV matmuls per block (512-free each, PSUM bank
    # limit); ACT exps the [128,1024] scores tile to bf16; DVE accumulates
    # the bf16 exp tiles into acc. At pass end two 128-wide ones-matmuls
    # turn acc into a partition-replicated denominator, DVE inverts it and
    # scales the AV accumulator straight out of PSUM.
    with (
        tc.tile_pool(name="sps", bufs=2, space="PSUM") as sps,  # 2 x [128,1024] = 4 banks
        tc.tile_pool(name="avp", bufs=1, space="PSUM") as avp,  # [128,1024] = 2 banks
        tc.tile_pool(name="rvp", bufs=1, space="PSUM") as rvp,  # [128,1024] = 2 banks
    ):
        for lq in range(NLQ):
            qs = slice(lq * 512, (lq + 1) * 512)
            for hp in range(2):  # head pairs
                p_idx = lq * 2 + hp
                psA = avp.tile([128, 1024], F32, tag="av", name="av")
                if p_idx in accs:
                    acc = accs[p_idx]
                    at_of = {lk: early_at[(p_idx, lk)] for lk in range(8)}
                    start_lk = 8
                else:
                    acc = accp.tile([128, 1024], BF16, tag="acc", name="acc")
                    at_of = {}
                    start_lk = 0

                def emit_sc(lk):
                    ss = sps.tile([128, 1024], F32, tag="sps", name="sps")
                    for j in range(2):
                        nc.tensor.matmul(
                            ss[:, j * 512:(j + 1) * 512],
                            lhsT=kT[:, lk * 128:(lk + 1) * 128],
                            rhs=qT[2 * hp + j][:, qs],
                            start=True,
                            stop=True,
                        )
                    at = attp.tile([128, 1024], BF16, tag="att", name="att")
                    nc.scalar.activation(at[:], ss[:], AF.Exp, scale=SCALE)
                    # softmax denominator partials on the DVE (2x bf16 rate)
                    if lk == 0:
                        nc.vector.tensor_copy(acc[:], at[:])
                    else:
                        nc.vector.tensor_add(acc[:], acc[:], at[:])
                    at_of[lk] = at

                def emit_av(lk):
                    for j in range(2):
                        nc.tensor.matmul(
                            psA[:, j * 512:(j + 1) * 512],
                            lhsT=vN[:, lk * 128:(lk + 1) * 128],
                            rhs=at_of[lk][:, j * 512:(j + 1) * 512],
                            start=lk == 0,
                            stop=lk == LKT - 1,
                        )

                if start_lk == 0:
                    # full pass: interleave, AV trailing exp by 2 blocks
                    pend = []
                    for lk in range(LKT):
                        emit_sc(lk)
                        pend.append(lk)
                        if len(pend) > 2:
                            emit_av(pend.pop(0))
                    while pend:
                        emit_av(pend.pop(0))
                else:
                    # first two passes: lk 0-7 already exp'd during the
                    # projections — stream their AVs while the missing
                    # scores trickle in at the ACT's exp cadence
                    emit_sc(8)
                    emit_sc(9)
                    next_sc = 10
                    for lk in range(LKT):
                        emit_av(lk)
                        if lk % 2 == 1 and next_sc < LKT:
                            emit_sc(next_sc)
                            next_sc += 1
                # partition-reduce + replicate the denominator: r[*, q] =
                # sum_k acc[k, q] for every output partition
                psR = rvp.tile([128, 1024], F32, tag="rv", name="rv")
                for j in range(2):
                    nc.tensor.matmul(
                        psR[:, j * 512:(j + 1) * 512],
                        lhsT=ones_bf[:],
                        rhs=acc[:, j * 512:(j + 1) * 512],
                        start=True,
                        stop=True,
                    )
                rinv = finp.tile([128, 1024], F32, tag="rinv", name="rinv")
                nc.vector.reciprocal_approx_fast(rinv[:], psR[:])
                ot = finp.tile([128, 1024], F32, tag="ot", name="ot")
                nc.vector.tensor_mul(ot[:], psA[:], rinv[:])
                for j in range(2):
                    h = 2 * hp + j
                    nc.sync.dma_start(
                        out=outT[h * 128:(h + 1) * 128, qs],
                        in_=ot[:, j * 512:(j + 1) * 512],
                    )


_NC_CACHE = None


def build_nc():
    global _NC_CACHE
    if _NC_CACHE is not None:
        return _NC_CACHE
    nc = bacc.Bacc("TRN2", target_bir_lowering=False, debug=False)
    xT = nc.dram_tensor("xT", [D, L], F32, kind="ExternalInput").ap()
    xkT = nc.dram_tensor("xkT", [D, 512], F32, kind="ExternalInput").ap()
    wT = nc.dram_tensor("wT", [D, QC + 2 * HD], F32, kind="ExternalInput").ap()
    bq = nc.dram_tensor("bq", [128, NH], F32, kind="ExternalInput").ap()
    bk = nc.dram_tensor("bk", [128, 1], F32, kind="ExternalInput").ap()
    bv = nc.dram_tensor("bv", [128, 1], F32, kind="ExternalInput").ap()
    outT = nc.dram_tensor("outT", [QC, L], F32, kind="ExternalOutput").ap()
    with tile.TileContext(nc) as tc, ExitStack() as ctx:
        build_kernel(ctx, tc, xT, xkT, wT, bq, bk, bv, outT)
    nc.compile()
    _NC_CACHE = nc
    return nc


def make_in_maps(x, Wq_w, Wq_b, Wk_w, Wk_b, Wv_w, Wv_b):
    """Host-side sharding/relayout. Returns one input map per core."""
    x = np.asarray(x, dtype=np.float32)
    Wq_w = np.asarray(Wq_w, dtype=np.float32)
    Wq_b = np.asarray(Wq_b, dtype=np.float32)
    Wk_w = np.asarray(Wk_w, dtype=np.float32)
    Wk_b = np.asarray(Wk_b, dtype=np.float32)
    Wv_w = np.asarray(Wv_w, dtype=np.float32)
    Wv_b = np.asarray(Wv_b, dtype=np.float32)

    xTs = [np.ascontiguousarray(x[b].T) for b in range(B)]
    wkvT = np.concatenate([Wk_w.T, Wv_w.T], axis=1)  # [D, 256]
    bk = np.ascontiguousarray(Wk_b.reshape(128, 1))
    bv = np.ascontiguousarray(Wv_b.reshape(128, 1))
    in_maps = []
    for c in range(N_CORES):
        b, g = divmod(c, B * 2)  # b = c // 4, g = c % 4
        # one contiguous [D, 768] weight tensor: [Wq_g | Wk | Wv].T
        wT_g = np.ascontiguousarray(
            np.concatenate([Wq_w[g * QC:(g + 1) * QC, :].T, wkvT], axis=1)
        )
        bq_g = np.ascontiguousarray(Wq_b[g * QC:(g + 1) * QC].reshape(NH, 128).T)
        in_maps.append(
            {
                "xT": xTs[b],
                "xkT": np.ascontiguousarray(xTs[b][:, g * 512:(g + 1) * 512]),
                "wT": wT_g,
                "bq": bq_g,
                "bk": bk,
                "bv": bv,
            }
        )
    return in_maps


def assemble_output(results):
    out = np.empty((B, L, D), dtype=np.float32)
    for c in range(N_CORES):
        b, g = divmod(c, B * 2)
        out[b, :, g * QC:(g + 1) * QC] = results[c]["outT"].T
    return out


def kernel(**inputs) -> np.ndarray:
    nc = build_nc()
    in_maps = make_in_maps(**inputs)
    res = run_bass_kernel_spmd(nc, in_maps, core_ids=list(range(N_CORES)))
    return assemble_output(res.results)
